# revision 11
# baseline (speedup 1.0000x reference)
"""Trainium2 Bass kernel for nn_Attention_73701638800011.

Reference computation (B=32, L=1024, H=1024):
    q = query @ W.T ; k = key @ W.T ; v = value @ W.T
    logits = relu(q @ w1.T + k @ w2.T + b)        # [B, L, 1]
    score  = softmax(logits, axis=-2)             # over L
    result = sum(score * v, axis=-2)              # [B, H]
    returns (result, score)

Algebraic collapse used here (exact up to fp reassociation):
    u1 = w1 @ W ; u2 = w2 @ W                     # [1, H] each (host, O(H^2))
    logits[b,l] = query[b,l,:]@u1 + key[b,l,:]@u2 + b
    e = exp(relu(logits)); score = e / sum_l e
    wv[b,:] = sum_l score[b,l] * value[b,l,:]     # contraction over L
    result = wv @ W.T
This removes all three O(B*L*H^2) projections; the device work is a single
streaming pass over query/key/value (DMA-bound) plus small matmuls.

Device mapping per core (4 samples):
  * q,k chunks DMA'd side by side into one tile; DVE multiplies by the
    replicated [u1|u2] row; ScalarE's activation accumulator reduces the
    2048-wide product to the per-row logit.  exp(relu(.)) via ScalarE with
    the e-sum accumulated for the softmax denominator.
  * wv via PE with value as the STATIONARY operand (fp32 moving operands
    stream at 1/4 rate, so the moving side is the [128,1] score column):
    out[,1] = v_chunk.T @ e_col accumulated over the 8 l-tiles.  1/Z is
    folded into the PSUM-evacuation copy, keeping Z off the critical path.
  * result likewise with W.T chunks stationary and the [128,4] wv block
    moving; the transposed [o,4] outputs are flipped back via PE transpose.

Sharding: data-parallel over batch B across the 8 cores (4 samples/core),
params replicated — per the problem's sharding hint.
"""

import numpy as np

import bass_rust
import concourse.bass as bass
import concourse.mybir as mybir
import concourse.tile as tile
from concourse.bass_utils import run_bass_kernel_spmd
from concourse.masks import make_identity

B, L, H = 32, 1024, 1024
NCORES = 8
BPC = B // NCORES  # samples per core
LT = L // 128      # l-tiles per sample
HC = H // 128      # h-chunks
XT = 2             # l-tiles loaded per DMA (1 MiB q + 1 MiB k per chunk)
ND = LT // XT

F32 = mybir.dt.float32


def _split_multi_waits(nc):
    """The walrus build in this container accepts at most ONE sync-wait per
    instruction ("Too many sync wait commands"), while Tile freely attaches
    several.  Semantically equivalent fix: move all but the last wait onto
    same-engine NoOps inserted immediately before the instruction (engines
    dispatch in program order, so a wait on a preceding NoOp gates the
    instruction identically)."""
    n = 0
    for f in nc.m.functions:
        for blk in f.blocks:
            out = []
            changed = False
            for inst in blk.instructions:
                si = inst.sync_info
                if si is not None and len(si.on_wait) > 1:
                    waits = list(si.on_wait)
                    for w in waits[:-1]:
                        nop = bass_rust.InstNoOp(
                            name=f"{inst.name}.sw{n}",
                            engine=inst.engine,
                            sync_info=mybir.SyncInfo(on_wait=[w], on_update=[]),
                        )
                        n += 1
                        out.append(nop)
                    inst.sync_info = mybir.SyncInfo(
                        on_wait=[waits[-1]], on_update=list(si.on_update))
                    changed = True
                out.append(inst)
            if changed:
                blk.instructions = out
    return n


def build_nc(reps: int = 1) -> bass.Bass:
    """reps > 1 statically replicates the whole computation inside one NEFF —
    used only for timing (slope between reps=1 and reps=R removes launch
    overhead)."""
    nc = bass.Bass()
    q = nc.declare_dram_parameter("q", [BPC, L, H], F32, isOutput=False)
    k = nc.declare_dram_parameter("k", [BPC, L, H], F32, isOutput=False)
    v = nc.declare_dram_parameter("v", [BPC, L, H], F32, isOutput=False)
    wt = nc.declare_dram_parameter("wt", [H, H], F32, isOutput=False)
    u12 = nc.declare_dram_parameter("u12", [1, 2 * H], F32, isOutput=False)
    bias = nc.declare_dram_parameter("bias", [1, 1], F32, isOutput=False)
    score = nc.declare_dram_parameter("score", [BPC, L], F32, isOutput=True)
    result = nc.declare_dram_parameter("result", [BPC, H], F32, isOutput=True)

    AF = mybir.ActivationFunctionType

    with tile.TileContext(nc) as tc, \
         tc.tile_pool(name="qkp", bufs=4) as qkp, \
         tc.tile_pool(name="vp", bufs=8) as vp, \
         tc.tile_pool(name="prodp", bufs=3) as prodp, \
         tc.tile_pool(name="singles", bufs=1) as singles, \
         tc.tile_pool(name="small", bufs=2) as small, \
         tc.tile_pool(name="ps_wvc", bufs=2, space="PSUM") as ps_wvc, \
         tc.tile_pool(name="ps_res", bufs=2, space="PSUM") as ps_res, \
         tc.tile_pool(name="ps_t", bufs=2, space="PSUM") as ps_t, \
         tc.tile_pool(name="ps_z", bufs=1, space="PSUM") as ps_z, \
         tc.tile_pool(name="ps_b", bufs=1, space="PSUM") as ps_b:

        # ---- constants / params on chip ----
        u12r = singles.tile([128, 2 * H], F32)
        nc.gpsimd.dma_start(out=u12r[:], in_=u12[:].broadcast_to([128, 2 * H]))
        biasr = singles.tile([128, 1], F32)
        nc.gpsimd.dma_start(out=biasr[:], in_=bias[:].broadcast_to([128, 1]))

        identity = singles.tile([128, 128], F32)
        make_identity(nc, identity[:])
        ones_k = singles.tile([128, 1], F32)
        nc.vector.memset(ones_k[:], 1.0)
        ones_m = singles.tile([1, 128], F32)
        nc.vector.memset(ones_m[:], 1.0)

        # W.T tiles [h_part, o]; loaded late (only the result phase needs it)
        wt_sb = singles.tile([128, HC * H], F32)
        wvt_sb = singles.tile([128, HC * BPC], F32)  # col c*BPC+b = wv[b, hc c]
        res_sb = singles.tile([BPC, H], F32)

        for _rep in range(reps):
            for b in range(BPC):
                # ---- logits: lg[p,t] = q[b,t*128+p,:]@u1 + k[...]@u2 ------
                lg = small.tile([128, LT], F32, tag="lg")
                vts = []
                for d in range(ND):
                    rows = slice(d * XT * 128, (d + 1) * XT * 128)
                    qk = qkp.tile([128, XT, 2 * H], F32)
                    nc.sync.dma_start(
                        out=qk[:, :, 0:H],
                        in_=q[b, rows, :].rearrange("(x p) h -> p x h", p=128))
                    nc.sync.dma_start(
                        out=qk[:, :, H:2 * H],
                        in_=k[b, rows, :].rearrange("(x p) h -> p x h", p=128))
                    vt = vp.tile([128, XT, H], F32)
                    nc.sync.dma_start(
                        out=vt[:], in_=v[b, rows, :].rearrange("(x p) h -> p x h", p=128))
                    vts.append(vt)
                    for x in range(XT):
                        t_idx = d * XT + x
                        prod = prodp.tile([128, 2 * H], F32)
                        nc.vector.tensor_mul(prod[:], qk[:, x, :], u12r[:])
                        nc.scalar.activation(
                            out=prod[:], in_=prod[:], func=AF.Copy,
                            accum_out=lg[:, t_idx:t_idx + 1])

                # ---- softmax pieces: e = exp(relu(lg + bias)) --------------
                lgr = small.tile([128, LT], F32, tag="lgr")
                nc.scalar.activation(out=lgr[:], in_=lg[:], func=AF.Relu,
                                     bias=biasr[:])
                e = small.tile([128, LT], F32, tag="e")
                esum = small.tile([128, 1], F32, tag="esum")
                nc.scalar.activation(out=e[:], in_=lgr[:], func=AF.Exp,
                                     accum_out=esum[:])
                # Z = sum(esum) across partitions; rzb = (1/Z) on all parts
                zps = ps_z.tile([1, 1], F32)
                nc.tensor.matmul(zps[:], lhsT=esum[:], rhs=ones_k[:],
                                 start=True, stop=True)
                rz = small.tile([1, 1], F32, tag="rz")
                nc.vector.reciprocal(rz[:], zps[:])
                rzbps = ps_b.tile([128, 1], F32)
                nc.tensor.matmul(rzbps[:], lhsT=ones_m[:], rhs=rz[:],
                                 start=True, stop=True)
                rzb = small.tile([128, 1], F32, tag="rzb")
                nc.vector.tensor_copy(rzb[:], rzbps[:])

                # ---- score output: transpose e, scale by 1/Z on the way ----
                scps = ps_t.tile([LT, 128], F32, tag="tr")
                nc.tensor.transpose(scps[:], e[:], identity[:])
                sc_sb = small.tile([LT, 128], F32, tag="sc")
                nc.scalar.activation(out=sc_sb[:], in_=scps[:], func=AF.Copy,
                                     scale=rzb[0:LT, :])
                nc.gpsimd.dma_start(
                    out=score[b:b + 1, :].rearrange("o (t p) -> (o t) p", p=128),
                    in_=sc_sb[:])

                # ---- wv[b, hc] via PE: v chunks stationary, e col moving ---
                # (fp32 moving operands stream at 1/4 rate; keep them [128,1])
                for c in range(HC):
                    cols = slice(c * 128, (c + 1) * 128)
                    wvc = ps_wvc.tile([128, 1], F32)
                    for t_idx in range(LT):
                        nc.tensor.matmul(
                            wvc[:],
                            lhsT=vts[t_idx // XT][:, t_idx % XT, cols],
                            rhs=e[:, t_idx:t_idx + 1],
                            start=(t_idx == 0), stop=(t_idx == LT - 1))
                    # evacuate with the 1/Z scale folded in
                    nc.vector.tensor_scalar_mul(
                        wvt_sb[:, c * BPC + b:c * BPC + b + 1],
                        in0=wvc[:], scalar1=rzb[:])

            # ---- result = wv @ W.T: wt chunks stationary, wv block moving --
            for c in range(HC):
                nc.sync.dma_start(out=wt_sb[:, c * H:(c + 1) * H],
                                  in_=wt[c * 128:(c + 1) * 128, :])
            for oc in range(HC):
                ocols = slice(oc * 128, (oc + 1) * 128)
                rps = ps_res.tile([128, BPC], F32)
                for hc in range(HC):
                    nc.tensor.matmul(
                        rps[:],
                        lhsT=wt_sb[:, hc * H + oc * 128: hc * H + (oc + 1) * 128],
                        rhs=wvt_sb[:, hc * BPC:(hc + 1) * BPC],
                        start=(hc == 0), stop=(hc == HC - 1))
                rt_sb = small.tile([128, BPC], F32, tag="rt")
                nc.vector.tensor_copy(rt_sb[:], rps[:])
                rtt = ps_t.tile([BPC, 128], F32, tag="tr")
                nc.tensor.transpose(rtt[:], rt_sb[:], identity[:])
                nc.scalar.copy(res_sb[:, ocols], rtt[:])
            nc.gpsimd.dma_start(out=result[:, :], in_=res_sb[:])

    _split_multi_waits(nc)
    return nc


_NC_CACHE = None


def _get_nc():
    global _NC_CACHE
    if _NC_CACHE is None:
        _NC_CACHE = build_nc()
    return _NC_CACHE


def kernel(query, key, value, W, mlp_w, mlp_b):
    query = np.ascontiguousarray(np.asarray(query, dtype=np.float32))
    key = np.ascontiguousarray(np.asarray(key, dtype=np.float32))
    value = np.ascontiguousarray(np.asarray(value, dtype=np.float32))
    W = np.ascontiguousarray(np.asarray(W, dtype=np.float32))
    mlp_w = np.asarray(mlp_w, dtype=np.float32)
    mlp_b = np.asarray(mlp_b, dtype=np.float32)

    # Host-side input prep (O(H^2), ~0.01% of the device work)
    W64 = W.astype(np.float64)
    u1 = (mlp_w[:, :H].astype(np.float64) @ W64).astype(np.float32)  # [1, H]
    u2 = (mlp_w[:, H:].astype(np.float64) @ W64).astype(np.float32)  # [1, H]
    u12 = np.ascontiguousarray(np.concatenate([u1, u2], axis=1))     # [1, 2H]
    wt = np.ascontiguousarray(W.T)                                   # [H, H]
    bias = mlp_b.reshape(1, 1)

    in_maps = []
    for i in range(NCORES):
        s = slice(i * BPC, (i + 1) * BPC)
        in_maps.append({
            "q": query[s], "k": key[s], "v": value[s],
            "wt": wt, "u12": u12, "bias": bias,
        })

    res = run_bass_kernel_spmd(_get_nc(), in_maps, core_ids=list(range(NCORES)))

    result = np.concatenate([r["result"] for r in res.results], axis=0)
    score = np.concatenate([r["score"] for r in res.results], axis=0)
    return result, score.reshape(B, L, 1)


# revision 16
# speedup vs baseline: 35.0610x; 35.0610x over previous
"""Trainium2 Bass kernel for nn_Attention_73701638800011.

Reference computation (B=32, L=1024, H=1024):
    q = query @ W.T ; k = key @ W.T ; v = value @ W.T
    logits = relu(q @ w1.T + k @ w2.T + b)        # [B, L, 1]
    score  = softmax(logits, axis=-2)             # over L
    result = sum(score * v, axis=-2)              # [B, H]
    returns (result, score)

Algebraic collapse used here (exact up to fp reassociation):
    u1 = w1 @ W ; u2 = w2 @ W                     # [1, H] each (host, O(H^2))
    logits[b,l] = query[b,l,:]@u1 + key[b,l,:]@u2 + b
    e = exp(relu(logits)); score = e / sum_l e
    wv[b,:] = sum_l score[b,l] * value[b,l,:]     # contraction over L
    result = wv @ W.T
This removes all three O(B*L*H^2) projections; the device work is a single
streaming pass over query/key/value (DMA-bound) plus small matmuls.

Device mapping per core (4 samples):
  * q,k chunks DMA'd side by side into one tile; DVE multiplies by the
    replicated [u1|u2] row; ScalarE's activation accumulator reduces the
    2048-wide product to the per-row logit.  exp(relu(.)) via ScalarE with
    the e-sum accumulated for the softmax denominator.
  * wv via PE with value as the STATIONARY operand (fp32 moving operands
    stream at 1/4 rate, so the moving side is the [128,1] score column):
    out[,1] = v_chunk.T @ e_col accumulated over the 8 l-tiles.  1/Z is
    folded into the PSUM-evacuation copy, keeping Z off the critical path.
  * result likewise with W.T chunks stationary and the [128,4] wv block
    moving; the transposed [o,4] outputs are flipped back via PE transpose.

Sharding: data-parallel over batch B across the 8 cores (4 samples/core),
params replicated — per the problem's sharding hint.
"""

import numpy as np

import bass_rust
import concourse.bass as bass
import concourse.mybir as mybir
import concourse.tile as tile
from concourse.bass_utils import run_bass_kernel_spmd
from concourse.masks import make_identity

B, L, H = 32, 1024, 1024
NCORES = 8
BPC = B // NCORES  # samples per core
LT = L // 128      # l-tiles per sample
HC = H // 128      # h-chunks
XT = 2             # l-tiles loaded per DMA (1 MiB q + 1 MiB k per chunk)
ND = LT // XT
VXT = 2            # l-tiles per v DMA
VND = LT // VXT

F32 = mybir.dt.float32
F16 = mybir.dt.float16

# HWDGE ring routing knobs (sync=SP ring, scalar=ACT ring)
V_DMA_ENGINE = "scalar"
K_DMA_ENGINE = "sync"


def _split_multi_waits(nc):
    """The walrus build in this container accepts at most ONE sync-wait per
    instruction ("Too many sync wait commands"), while Tile freely attaches
    several.  Semantically equivalent fix: move all but the last wait onto
    same-engine NoOps inserted immediately before the instruction (engines
    dispatch in program order, so a wait on a preceding NoOp gates the
    instruction identically)."""
    n = 0
    for f in nc.m.functions:
        for blk in f.blocks:
            out = []
            changed = False
            for inst in blk.instructions:
                si = inst.sync_info
                if si is not None and len(si.on_wait) > 1:
                    waits = list(si.on_wait)
                    for w in waits[:-1]:
                        nop = bass_rust.InstNoOp(
                            name=f"{inst.name}.sw{n}",
                            engine=inst.engine,
                            sync_info=mybir.SyncInfo(on_wait=[w], on_update=[]),
                        )
                        n += 1
                        out.append(nop)
                    inst.sync_info = mybir.SyncInfo(
                        on_wait=[waits[-1]], on_update=list(si.on_update))
                    changed = True
                out.append(inst)
            if changed:
                blk.instructions = out
    return n


def build_nc(reps: int = 1) -> bass.Bass:
    """reps > 1 statically replicates the whole computation inside one NEFF —
    used only for timing (slope between reps=1 and reps=R removes launch
    overhead)."""
    nc = bass.Bass()
    q = nc.declare_dram_parameter("q", [BPC, L, H], F16, isOutput=False)
    k = nc.declare_dram_parameter("k", [BPC, L, H], F16, isOutput=False)
    v = nc.declare_dram_parameter("v", [BPC, L, H], F16, isOutput=False)
    wt = nc.declare_dram_parameter("wt", [H, H], F32, isOutput=False)
    u12 = nc.declare_dram_parameter("u12", [1, 2 * H], F32, isOutput=False)
    bias = nc.declare_dram_parameter("bias", [1, 1], F32, isOutput=False)
    score = nc.declare_dram_parameter("score", [BPC, L], F32, isOutput=True)
    result = nc.declare_dram_parameter("result", [BPC, H], F32, isOutput=True)

    AF = mybir.ActivationFunctionType

    with tile.TileContext(nc) as tc, \
         tc.tile_pool(name="qkp", bufs=4) as qkp, \
         tc.tile_pool(name="vp", bufs=8) as vp, \
         tc.tile_pool(name="prodp", bufs=3) as prodp, \
         tc.tile_pool(name="singles", bufs=1) as singles, \
         tc.tile_pool(name="small", bufs=2) as small, \
         tc.tile_pool(name="ps_wvc", bufs=2, space="PSUM") as ps_wvc, \
         tc.tile_pool(name="ps_res", bufs=2, space="PSUM") as ps_res, \
         tc.tile_pool(name="ps_t", bufs=2, space="PSUM") as ps_t, \
         tc.tile_pool(name="ps_z", bufs=1, space="PSUM") as ps_z, \
         tc.tile_pool(name="ps_b", bufs=1, space="PSUM") as ps_b:

        # ---- constants / params on chip ----
        u12r = singles.tile([128, 2 * H], F32)
        nc.gpsimd.dma_start(out=u12r[:], in_=u12[:].broadcast_to([128, 2 * H]))
        biasr = singles.tile([128, 1], F32)
        nc.gpsimd.dma_start(out=biasr[:], in_=bias[:].broadcast_to([128, 1]))

        identity = singles.tile([128, 128], F32)
        make_identity(nc, identity[:])
        ones_k = singles.tile([128, 1], F32)
        nc.vector.memset(ones_k[:], 1.0)
        ones_m = singles.tile([1, 128], F32)
        nc.vector.memset(ones_m[:], 1.0)

        # W.T tiles [h_part, o]; loaded late (only the result phase needs it)
        wt_sb = singles.tile([128, HC * H], F32)
        wvt_sb = singles.tile([128, HC * BPC], F32)  # col c*BPC+b = wv[b, hc c]
        res_sb = singles.tile([BPC, H], F32)

        for _rep in range(reps):
            for b in range(BPC):
                # ---- logits: lg[p,t] = q[b,t*128+p,:]@u1 + k[...]@u2 ------
                lg = small.tile([128, LT], F32, tag="lg")
                for d in range(ND):
                    rows = slice(d * XT * 128, (d + 1) * XT * 128)
                    qk = qkp.tile([128, XT, 2 * H], F16)
                    nc.sync.dma_start(
                        out=qk[:, :, 0:H],
                        in_=q[b, rows, :].rearrange("(x p) h -> p x h", p=128))
                    getattr(nc, K_DMA_ENGINE).dma_start(
                        out=qk[:, :, H:2 * H],
                        in_=k[b, rows, :].rearrange("(x p) h -> p x h", p=128))
                    for x in range(XT):
                        t_idx = d * XT + x
                        prod = prodp.tile([128, 2 * H], F32)
                        nc.vector.tensor_mul(prod[:], qk[:, x, :], u12r[:])
                        nc.scalar.activation(
                            out=prod[:], in_=prod[:], func=AF.Copy,
                            accum_out=lg[:, t_idx:t_idx + 1])

                vts = []
                for vd in range(VND):
                    vrows = slice(vd * VXT * 128, (vd + 1) * VXT * 128)
                    vt = vp.tile([128, VXT, H], F16)
                    getattr(nc, V_DMA_ENGINE).dma_start(
                        out=vt[:], in_=v[b, vrows, :].rearrange("(x p) h -> p x h", p=128))
                    vts.append(vt)

                # ---- softmax pieces: e = exp(relu(lg + bias)) --------------
                lgr = small.tile([128, LT], F32, tag="lgr")
                nc.scalar.activation(out=lgr[:], in_=lg[:], func=AF.Relu,
                                     bias=biasr[:])
                e = small.tile([128, LT], F32, tag="e")
                esum = small.tile([128, 1], F32, tag="esum")
                nc.scalar.activation(out=e[:], in_=lgr[:], func=AF.Exp,
                                     accum_out=esum[:])
                e_h = small.tile([128, LT], F16, tag="eh")
                nc.vector.tensor_copy(e_h[:], e[:])
                # Z = sum(esum) across partitions; rzb = (1/Z) on all parts
                zps = ps_z.tile([1, 1], F32)
                nc.tensor.matmul(zps[:], lhsT=esum[:], rhs=ones_k[:],
                                 start=True, stop=True)
                rz = small.tile([1, 1], F32, tag="rz")
                nc.vector.reciprocal(rz[:], zps[:])
                rzbps = ps_b.tile([128, 1], F32)
                nc.tensor.matmul(rzbps[:], lhsT=ones_m[:], rhs=rz[:],
                                 start=True, stop=True)
                rzb = small.tile([128, 1], F32, tag="rzb")
                nc.vector.tensor_copy(rzb[:], rzbps[:])

                # ---- score output: transpose e, scale by 1/Z on the way ----
                scps = ps_t.tile([LT, 128], F32, tag="tr")
                nc.tensor.transpose(scps[:], e[:], identity[:])
                sc_sb = small.tile([LT, 128], F32, tag="sc")
                nc.scalar.activation(out=sc_sb[:], in_=scps[:], func=AF.Copy,
                                     scale=rzb[0:LT, :])
                nc.gpsimd.dma_start(
                    out=score[b:b + 1, :].rearrange("o (t p) -> (o t) p", p=128),
                    in_=sc_sb[:])

                # ---- wv[b, hc] via PE: v chunks stationary, e col moving ---
                # (fp32 moving operands stream at 1/4 rate; keep them [128,1])
                for c in range(HC):
                    cols = slice(c * 128, (c + 1) * 128)
                    wvc = ps_wvc.tile([128, 1], F32)
                    for t_idx in range(LT):
                        nc.tensor.matmul(
                            wvc[:],
                            lhsT=vts[t_idx // VXT][:, t_idx % VXT, cols],
                            rhs=e_h[:, t_idx:t_idx + 1],
                            start=(t_idx == 0), stop=(t_idx == LT - 1))
                    # evacuate with the 1/Z scale folded in
                    nc.vector.tensor_scalar_mul(
                        wvt_sb[:, c * BPC + b:c * BPC + b + 1],
                        in0=wvc[:], scalar1=rzb[:])

            # ---- result = wv @ W.T: wt chunks stationary, wv block moving --
            for c in range(HC):
                nc.sync.dma_start(out=wt_sb[:, c * H:(c + 1) * H],
                                  in_=wt[c * 128:(c + 1) * 128, :])
            for oc in range(HC):
                ocols = slice(oc * 128, (oc + 1) * 128)
                rps = ps_res.tile([128, BPC], F32)
                for hc in range(HC):
                    nc.tensor.matmul(
                        rps[:],
                        lhsT=wt_sb[:, hc * H + oc * 128: hc * H + (oc + 1) * 128],
                        rhs=wvt_sb[:, hc * BPC:(hc + 1) * BPC],
                        start=(hc == 0), stop=(hc == HC - 1))
                rt_sb = small.tile([128, BPC], F32, tag="rt")
                nc.vector.tensor_copy(rt_sb[:], rps[:])
                rtt = ps_t.tile([BPC, 128], F32, tag="tr")
                nc.tensor.transpose(rtt[:], rt_sb[:], identity[:])
                nc.scalar.copy(res_sb[:, ocols], rtt[:])
            nc.gpsimd.dma_start(out=result[:, :], in_=res_sb[:])

    _split_multi_waits(nc)
    return nc


_NC_CACHE = None


def _get_nc():
    global _NC_CACHE
    if _NC_CACHE is None:
        _NC_CACHE = build_nc()
    return _NC_CACHE


def kernel(query, key, value, W, mlp_w, mlp_b):
    # q/k/v travel to the device as fp16 (|x| ~ N(0,1), well inside fp16
    # range; adds ~3e-4 scale-relative absmax error, halves the DMA traffic
    # that dominates this kernel).  All accumulation stays fp32 on device.
    query = np.ascontiguousarray(np.asarray(query).astype(np.float16))
    key = np.ascontiguousarray(np.asarray(key).astype(np.float16))
    value = np.ascontiguousarray(np.asarray(value).astype(np.float16))
    W = np.ascontiguousarray(np.asarray(W, dtype=np.float32))
    mlp_w = np.asarray(mlp_w, dtype=np.float32)
    mlp_b = np.asarray(mlp_b, dtype=np.float32)

    # Host-side input prep (O(H^2), ~0.01% of the device work)
    W64 = W.astype(np.float64)
    u1 = (mlp_w[:, :H].astype(np.float64) @ W64).astype(np.float32)  # [1, H]
    u2 = (mlp_w[:, H:].astype(np.float64) @ W64).astype(np.float32)  # [1, H]
    u12 = np.ascontiguousarray(np.concatenate([u1, u2], axis=1))     # [1, 2H]
    wt = np.ascontiguousarray(W.T)                                   # [H, H]
    bias = mlp_b.reshape(1, 1)

    in_maps = []
    for i in range(NCORES):
        s = slice(i * BPC, (i + 1) * BPC)
        in_maps.append({
            "q": query[s], "k": key[s], "v": value[s],
            "wt": wt, "u12": u12, "bias": bias,
        })

    res = run_bass_kernel_spmd(_get_nc(), in_maps, core_ids=list(range(NCORES)))

    result = np.concatenate([r["result"] for r in res.results], axis=0)
    score = np.concatenate([r["score"] for r in res.results], axis=0)
    return result, score.reshape(B, L, 1)


# revision 21
# speedup vs baseline: 47.4778x; 1.3541x over previous
"""Trainium2 Bass kernel for nn_Attention_73701638800011.

Reference computation (B=32, L=1024, H=1024):
    q = query @ W.T ; k = key @ W.T ; v = value @ W.T
    logits = relu(q @ w1.T + k @ w2.T + b)        # [B, L, 1]
    score  = softmax(logits, axis=-2)             # over L
    result = sum(score * v, axis=-2)              # [B, H]
    returns (result, score)

Algebraic collapse used here (exact up to fp reassociation):
    u1 = w1 @ W ; u2 = w2 @ W                     # [1, H] each (host, O(H^2))
    logits[b,l] = query[b,l,:]@u1 + key[b,l,:]@u2 + b
    e = exp(relu(logits)); score = e / sum_l e
    wv[b,:] = sum_l score[b,l] * value[b,l,:]     # contraction over L
    result = wv @ W.T
This removes all three O(B*L*H^2) projections; the device work is a single
streaming pass over query/key/value (DMA-bound) plus small matmuls.

Device mapping per core (4 samples):
  * q,k chunks DMA'd side by side into one tile; DVE multiplies by the
    replicated [u1|u2] row; ScalarE's activation accumulator reduces the
    2048-wide product to the per-row logit.  exp(relu(.)) via ScalarE with
    the e-sum accumulated for the softmax denominator.
  * wv via PE with value as the STATIONARY operand (fp32 moving operands
    stream at 1/4 rate, so the moving side is the [128,1] score column):
    out[,1] = v_chunk.T @ e_col accumulated over the 8 l-tiles.  1/Z is
    folded into the PSUM-evacuation copy, keeping Z off the critical path.
  * result likewise with W.T chunks stationary and the [128,4] wv block
    moving; the transposed [o,4] outputs are flipped back via PE transpose.

Sharding: data-parallel over batch B across the 8 cores (4 samples/core),
params replicated — per the problem's sharding hint.
"""

import numpy as np

import bass_rust
import concourse.bass as bass
import concourse.mybir as mybir
import concourse.tile as tile
from concourse.bass_utils import run_bass_kernel_spmd
from concourse.masks import make_identity

B, L, H = 32, 1024, 1024
NCORES = 8
BPC = B // NCORES  # samples per core
LT = L // 128      # l-tiles per sample
HC = H // 128      # h-chunks
XT = 2             # l-tiles loaded per DMA (1 MiB q + 1 MiB k per chunk)
ND = LT // XT
VXT = 2            # l-tiles per v DMA
VND = LT // VXT

F32 = mybir.dt.float32
F16 = mybir.dt.float16

# HWDGE ring routing knobs (sync=SP ring, scalar=ACT ring)
V_DMA_ENGINE = "scalar"
K_DMA_ENGINE = "sync"


def _split_multi_waits(nc):
    """The walrus build in this container accepts at most ONE sync-wait per
    instruction ("Too many sync wait commands"), while Tile freely attaches
    several.  Semantically equivalent fix: move all but the last wait onto
    same-engine NoOps inserted immediately before the instruction (engines
    dispatch in program order, so a wait on a preceding NoOp gates the
    instruction identically)."""
    n = 0
    for f in nc.m.functions:
        for blk in f.blocks:
            out = []
            changed = False
            for inst in blk.instructions:
                si = inst.sync_info
                if si is not None and len(si.on_wait) > 1:
                    waits = list(si.on_wait)
                    for w in waits[:-1]:
                        nop = bass_rust.InstNoOp(
                            name=f"{inst.name}.sw{n}",
                            engine=inst.engine,
                            sync_info=mybir.SyncInfo(on_wait=[w], on_update=[]),
                        )
                        n += 1
                        out.append(nop)
                    inst.sync_info = mybir.SyncInfo(
                        on_wait=[waits[-1]], on_update=list(si.on_update))
                    changed = True
                out.append(inst)
            if changed:
                blk.instructions = out
    return n


def build_nc(reps: int = 1) -> bass.Bass:
    """reps > 1 statically replicates the whole computation inside one NEFF —
    used only for timing (slope between reps=1 and reps=R removes launch
    overhead)."""
    nc = bass.Bass()
    q = nc.declare_dram_parameter("q", [BPC, L, H], F16, isOutput=False)
    k = nc.declare_dram_parameter("k", [BPC, L, H], F16, isOutput=False)
    v = nc.declare_dram_parameter("v", [BPC, L, H], F16, isOutput=False)
    wt = nc.declare_dram_parameter("wt", [H, H], F32, isOutput=False)
    u12 = nc.declare_dram_parameter("u12", [1, 2 * H], F32, isOutput=False)
    bias = nc.declare_dram_parameter("bias", [1, 1], F32, isOutput=False)
    score = nc.declare_dram_parameter("score", [BPC, L], F32, isOutput=True)
    result = nc.declare_dram_parameter("result", [BPC, H], F32, isOutput=True)

    AF = mybir.ActivationFunctionType

    with tile.TileContext(nc) as tc, \
         tc.tile_pool(name="qkp", bufs=6) as qkp, \
         tc.tile_pool(name="vp", bufs=12) as vp, \
         tc.tile_pool(name="prodp", bufs=4) as prodp, \
         tc.tile_pool(name="singles", bufs=1) as singles, \
         tc.tile_pool(name="small", bufs=2) as small, \
         tc.tile_pool(name="ps_wvc", bufs=2, space="PSUM") as ps_wvc, \
         tc.tile_pool(name="ps_res", bufs=2, space="PSUM") as ps_res, \
         tc.tile_pool(name="ps_t", bufs=2, space="PSUM") as ps_t, \
         tc.tile_pool(name="ps_z", bufs=1, space="PSUM") as ps_z, \
         tc.tile_pool(name="ps_b", bufs=1, space="PSUM") as ps_b:

        # ---- constants / params on chip ----
        u12r = singles.tile([128, 2 * H], F32)
        nc.gpsimd.dma_start(out=u12r[:], in_=u12[:].broadcast_to([128, 2 * H]))
        biasr = singles.tile([128, 1], F32)
        nc.gpsimd.dma_start(out=biasr[:], in_=bias[:].broadcast_to([128, 1]))

        identity = singles.tile([128, 128], F32)
        make_identity(nc, identity[:])
        ones_k = singles.tile([128, 1], F32)
        nc.vector.memset(ones_k[:], 1.0)
        ones_m = singles.tile([1, 128], F32)
        nc.vector.memset(ones_m[:], 1.0)

        # W.T tiles [h_part, o]; loaded late (only the result phase needs it)
        wt_sb = singles.tile([128, HC * H], F32)
        wvt_sb = singles.tile([128, HC * BPC], F32)  # col c*BPC+b = wv[b, hc c]
        res_sb = singles.tile([BPC, H], F32)

        for _rep in range(reps):
            for b in range(BPC):
                # ---- logits: lg[p,t] = q[b,t*128+p,:]@u1 + k[...]@u2 ------
                lg = small.tile([128, LT], F32, tag="lg")
                for d in range(ND):
                    rows = slice(d * XT * 128, (d + 1) * XT * 128)
                    qk = qkp.tile([128, XT, 2 * H], F16)
                    nc.sync.dma_start(
                        out=qk[:, :, 0:H],
                        in_=q[b, rows, :].rearrange("(x p) h -> p x h", p=128))
                    getattr(nc, K_DMA_ENGINE).dma_start(
                        out=qk[:, :, H:2 * H],
                        in_=k[b, rows, :].rearrange("(x p) h -> p x h", p=128))
                    for x in range(XT):
                        t_idx = d * XT + x
                        prod = prodp.tile([128, 2 * H], F32)
                        nc.vector.tensor_mul(prod[:], qk[:, x, :], u12r[:])
                        nc.scalar.activation(
                            out=prod[:], in_=prod[:], func=AF.Copy,
                            accum_out=lg[:, t_idx:t_idx + 1])

                vts = []
                for vd in range(VND):
                    vrows = slice(vd * VXT * 128, (vd + 1) * VXT * 128)
                    vt = vp.tile([128, VXT, H], F16)
                    getattr(nc, V_DMA_ENGINE).dma_start(
                        out=vt[:], in_=v[b, vrows, :].rearrange("(x p) h -> p x h", p=128))
                    vts.append(vt)

                # ---- softmax pieces: e = exp(relu(lg + bias)) --------------
                lgr = small.tile([128, LT], F32, tag="lgr")
                nc.scalar.activation(out=lgr[:], in_=lg[:], func=AF.Relu,
                                     bias=biasr[:])
                e = small.tile([128, LT], F32, tag="e")
                esum = small.tile([128, 1], F32, tag="esum")
                nc.scalar.activation(out=e[:], in_=lgr[:], func=AF.Exp,
                                     accum_out=esum[:])
                e_h = small.tile([128, LT], F16, tag="eh")
                nc.vector.tensor_copy(e_h[:], e[:])
                # Z = sum(esum) across partitions; rzb = (1/Z) on all parts
                zps = ps_z.tile([1, 1], F32)
                nc.tensor.matmul(zps[:], lhsT=esum[:], rhs=ones_k[:],
                                 start=True, stop=True)
                rz = small.tile([1, 1], F32, tag="rz")
                nc.vector.reciprocal(rz[:], zps[:])
                rzbps = ps_b.tile([128, 1], F32)
                nc.tensor.matmul(rzbps[:], lhsT=ones_m[:], rhs=rz[:],
                                 start=True, stop=True)
                rzb = small.tile([128, 1], F32, tag="rzb")
                nc.vector.tensor_copy(rzb[:], rzbps[:])

                # ---- score output: transpose e, scale by 1/Z on the way ----
                scps = ps_t.tile([LT, 128], F32, tag="tr")
                nc.tensor.transpose(scps[:], e[:], identity[:])
                sc_sb = small.tile([LT, 128], F32, tag="sc")
                nc.scalar.activation(out=sc_sb[:], in_=scps[:], func=AF.Copy,
                                     scale=rzb[0:LT, :])
                nc.gpsimd.dma_start(
                    out=score[b:b + 1, :].rearrange("o (t p) -> (o t) p", p=128),
                    in_=sc_sb[:])

                # ---- wv[b, hc] via PE: v chunks stationary, e col moving ---
                # (fp32 moving operands stream at 1/4 rate; keep them [128,1])
                for c in range(HC):
                    cols = slice(c * 128, (c + 1) * 128)
                    wvc = ps_wvc.tile([128, 1], F32)
                    for t_idx in range(LT):
                        nc.tensor.matmul(
                            wvc[:],
                            lhsT=vts[t_idx // VXT][:, t_idx % VXT, cols],
                            rhs=e_h[:, t_idx:t_idx + 1],
                            start=(t_idx == 0), stop=(t_idx == LT - 1))
                    # evacuate with the 1/Z scale folded in
                    nc.vector.tensor_scalar_mul(
                        wvt_sb[:, c * BPC + b:c * BPC + b + 1],
                        in0=wvc[:], scalar1=rzb[:])

            # ---- result = wv @ W.T: wt chunks stationary, wv block moving --
            for c in range(HC):
                nc.sync.dma_start(out=wt_sb[:, c * H:(c + 1) * H],
                                  in_=wt[c * 128:(c + 1) * 128, :])
            for oc in range(HC):
                ocols = slice(oc * 128, (oc + 1) * 128)
                rps = ps_res.tile([128, BPC], F32)
                for hc in range(HC):
                    nc.tensor.matmul(
                        rps[:],
                        lhsT=wt_sb[:, hc * H + oc * 128: hc * H + (oc + 1) * 128],
                        rhs=wvt_sb[:, hc * BPC:(hc + 1) * BPC],
                        start=(hc == 0), stop=(hc == HC - 1))
                rt_sb = small.tile([128, BPC], F32, tag="rt")
                nc.vector.tensor_copy(rt_sb[:], rps[:])
                rtt = ps_t.tile([BPC, 128], F32, tag="tr")
                nc.tensor.transpose(rtt[:], rt_sb[:], identity[:])
                nc.scalar.copy(res_sb[:, ocols], rtt[:])
            nc.gpsimd.dma_start(out=result[:, :], in_=res_sb[:])

    _split_multi_waits(nc)
    return nc


_NC_CACHE = None


def _get_nc():
    global _NC_CACHE
    if _NC_CACHE is None:
        _NC_CACHE = build_nc()
    return _NC_CACHE


def kernel(query, key, value, W, mlp_w, mlp_b):
    # q/k/v travel to the device as fp16 (|x| ~ N(0,1), well inside fp16
    # range; adds ~3e-4 scale-relative absmax error, halves the DMA traffic
    # that dominates this kernel).  All accumulation stays fp32 on device.
    query = np.ascontiguousarray(np.asarray(query).astype(np.float16))
    key = np.ascontiguousarray(np.asarray(key).astype(np.float16))
    value = np.ascontiguousarray(np.asarray(value).astype(np.float16))
    W = np.ascontiguousarray(np.asarray(W, dtype=np.float32))
    mlp_w = np.asarray(mlp_w, dtype=np.float32)
    mlp_b = np.asarray(mlp_b, dtype=np.float32)

    # Host-side input prep (O(H^2), ~0.01% of the device work)
    W64 = W.astype(np.float64)
    u1 = (mlp_w[:, :H].astype(np.float64) @ W64).astype(np.float32)  # [1, H]
    u2 = (mlp_w[:, H:].astype(np.float64) @ W64).astype(np.float32)  # [1, H]
    u12 = np.ascontiguousarray(np.concatenate([u1, u2], axis=1))     # [1, 2H]
    wt = np.ascontiguousarray(W.T)                                   # [H, H]
    bias = mlp_b.reshape(1, 1)

    in_maps = []
    for i in range(NCORES):
        s = slice(i * BPC, (i + 1) * BPC)
        in_maps.append({
            "q": query[s], "k": key[s], "v": value[s],
            "wt": wt, "u12": u12, "bias": bias,
        })

    res = run_bass_kernel_spmd(_get_nc(), in_maps, core_ids=list(range(NCORES)))

    result = np.concatenate([r["result"] for r in res.results], axis=0)
    score = np.concatenate([r["score"] for r in res.results], axis=0)
    return result, score.reshape(B, L, 1)


# revision 24
# speedup vs baseline: 95.4031x; 2.0094x over previous
"""Trainium2 Bass kernel for nn_Attention_73701638800011.

Reference computation (B=32, L=1024, H=1024):
    q = query @ W.T ; k = key @ W.T ; v = value @ W.T
    logits = relu(q @ w1.T + k @ w2.T + b)        # [B, L, 1]
    score  = softmax(logits, axis=-2)             # over L
    result = sum(score * v, axis=-2)              # [B, H]
    returns (result, score)

Algebraic collapse used here (exact up to fp reassociation):
    u1 = w1 @ W ; u2 = w2 @ W                     # [1, H] each (host, O(H^2))
    logits[b,l] = query[b,l,:]@u1 + key[b,l,:]@u2 + b
    e = exp(relu(logits)); score = e / sum_l e
    wv[b,:] = sum_l score[b,l] * value[b,l,:]     # contraction over L
    result = wv @ W.T
This removes all three O(B*L*H^2) projections; the device work is a single
streaming pass over query/key/value (DMA-bound) plus small matmuls.

Device mapping per core (4 samples):
  * q,k chunks DMA'd side by side into one tile; DVE multiplies by the
    replicated [u1|u2] row; ScalarE's activation accumulator reduces the
    2048-wide product to the per-row logit.  exp(relu(.)) via ScalarE with
    the e-sum accumulated for the softmax denominator.
  * wv via PE with value as the STATIONARY operand (fp32 moving operands
    stream at 1/4 rate, so the moving side is the [128,1] score column):
    out[,1] = v_chunk.T @ e_col accumulated over the 8 l-tiles.  1/Z is
    folded into the PSUM-evacuation copy, keeping Z off the critical path.
  * result likewise with W.T chunks stationary and the [128,4] wv block
    moving; the transposed [o,4] outputs are flipped back via PE transpose.

Sharding: data-parallel over batch B across the 8 cores (4 samples/core),
params replicated — per the problem's sharding hint.
"""

import numpy as np

import bass_rust
import concourse.bass as bass
import concourse.mybir as mybir
import concourse.tile as tile
from concourse.bass_utils import run_bass_kernel_spmd
from concourse.masks import make_identity

B, L, H = 32, 1024, 1024
NCORES = 8
BPC = B // NCORES  # samples per core
LT = L // 128      # l-tiles per sample
HC = H // 128      # h-chunks
XT = 2             # l-tiles loaded per DMA (1 MiB q + 1 MiB k per chunk)
ND = LT // XT
VXT = 2            # l-tiles per v DMA
VND = LT // VXT

F32 = mybir.dt.float32
F16 = mybir.dt.float16

# HWDGE ring routing knobs (sync=SP ring, scalar=ACT ring)
V_DMA_ENGINE = "scalar"
K_DMA_ENGINE = "sync"


def _split_multi_waits(nc):
    """The walrus build in this container accepts at most ONE sync-wait per
    instruction ("Too many sync wait commands"), while Tile freely attaches
    several.  Semantically equivalent fix: move all but the last wait onto
    same-engine NoOps inserted immediately before the instruction (engines
    dispatch in program order, so a wait on a preceding NoOp gates the
    instruction identically)."""
    n = 0
    for f in nc.m.functions:
        for blk in f.blocks:
            out = []
            changed = False
            for inst in blk.instructions:
                si = inst.sync_info
                if si is not None and len(si.on_wait) > 1:
                    waits = list(si.on_wait)
                    for w in waits[:-1]:
                        nop = bass_rust.InstNoOp(
                            name=f"{inst.name}.sw{n}",
                            engine=inst.engine,
                            sync_info=mybir.SyncInfo(on_wait=[w], on_update=[]),
                        )
                        n += 1
                        out.append(nop)
                    inst.sync_info = mybir.SyncInfo(
                        on_wait=[waits[-1]], on_update=list(si.on_update))
                    changed = True
                out.append(inst)
            if changed:
                blk.instructions = out
    return n


def build_nc(reps: int = 1) -> bass.Bass:
    """reps > 1 statically replicates the whole computation inside one NEFF —
    used only for timing (slope between reps=1 and reps=R removes launch
    overhead)."""
    nc = bass.Bass()
    q = nc.declare_dram_parameter("q", [BPC, L, H], F16, isOutput=False)
    k = nc.declare_dram_parameter("k", [BPC, L, H], F16, isOutput=False)
    v = nc.declare_dram_parameter("v", [BPC, L, H], F16, isOutput=False)
    wt = nc.declare_dram_parameter("wt", [H, H], F16, isOutput=False)
    u12 = nc.declare_dram_parameter("u12", [1, 2 * H], F32, isOutput=False)
    bias = nc.declare_dram_parameter("bias", [1, 1], F32, isOutput=False)
    score = nc.declare_dram_parameter("score", [BPC, L], F32, isOutput=True)
    result = nc.declare_dram_parameter("result", [BPC, H], F32, isOutput=True)

    AF = mybir.ActivationFunctionType

    with tile.TileContext(nc) as tc, \
         tc.tile_pool(name="qkp", bufs=6) as qkp, \
         tc.tile_pool(name="vp", bufs=12) as vp, \
         tc.tile_pool(name="prodp", bufs=4) as prodp, \
         tc.tile_pool(name="singles", bufs=1) as singles, \
         tc.tile_pool(name="small", bufs=2) as small, \
         tc.tile_pool(name="ps_wvc", bufs=2, space="PSUM") as ps_wvc, \
         tc.tile_pool(name="ps_res", bufs=1, space="PSUM") as ps_res, \
         tc.tile_pool(name="ps_t", bufs=2, space="PSUM") as ps_t, \
         tc.tile_pool(name="ps_z", bufs=1, space="PSUM") as ps_z, \
         tc.tile_pool(name="ps_b", bufs=1, space="PSUM") as ps_b:

        # ---- constants / params on chip ----
        u12r = singles.tile([128, 2 * H], F32)
        nc.gpsimd.dma_start(out=u12r[:], in_=u12[:].broadcast_to([128, 2 * H]))
        biasr = singles.tile([128, 1], F32)
        nc.gpsimd.dma_start(out=biasr[:], in_=bias[:].broadcast_to([128, 1]))

        identity = singles.tile([128, 128], F32)
        make_identity(nc, identity[:])
        ones_k = singles.tile([128, 1], F32)
        nc.vector.memset(ones_k[:], 1.0)
        ones_m = singles.tile([1, 128], F32)
        nc.vector.memset(ones_m[:], 1.0)

        # W.T tiles [h_part, o]; loaded late (only the result phase needs it)
        wt_sb = singles.tile([128, HC * H], F16)
        wvt_sb = singles.tile([128, HC * BPC], F16)  # col c*BPC+b = wv[b, hc c]
        res_sb = singles.tile([BPC, H], F32)

        for _rep in range(reps):
            for b in range(BPC):
                # ---- logits: lg[p,t] = q[b,t*128+p,:]@u1 + k[...]@u2 ------
                lg = small.tile([128, LT], F32, tag="lg")
                for d in range(ND):
                    rows = slice(d * XT * 128, (d + 1) * XT * 128)
                    qk = qkp.tile([128, XT, 2 * H], F16)
                    nc.sync.dma_start(
                        out=qk[:, :, 0:H],
                        in_=q[b, rows, :].rearrange("(x p) h -> p x h", p=128))
                    getattr(nc, K_DMA_ENGINE).dma_start(
                        out=qk[:, :, H:2 * H],
                        in_=k[b, rows, :].rearrange("(x p) h -> p x h", p=128))
                    for x in range(XT):
                        t_idx = d * XT + x
                        prod = prodp.tile([128, 2 * H], F32)
                        nc.vector.tensor_mul(prod[:], qk[:, x, :], u12r[:])
                        nc.scalar.activation(
                            out=prod[:], in_=prod[:], func=AF.Copy,
                            accum_out=lg[:, t_idx:t_idx + 1])

                vts = []
                for vd in range(VND):
                    vrows = slice(vd * VXT * 128, (vd + 1) * VXT * 128)
                    vt = vp.tile([128, VXT, H], F16)
                    getattr(nc, V_DMA_ENGINE).dma_start(
                        out=vt[:], in_=v[b, vrows, :].rearrange("(x p) h -> p x h", p=128))
                    vts.append(vt)

                # ---- softmax pieces: e = exp(relu(lg + bias)) --------------
                lgr = small.tile([128, LT], F32, tag="lgr")
                nc.scalar.activation(out=lgr[:], in_=lg[:], func=AF.Relu,
                                     bias=biasr[:])
                e = small.tile([128, LT], F32, tag="e")
                esum = small.tile([128, 1], F32, tag="esum")
                nc.scalar.activation(out=e[:], in_=lgr[:], func=AF.Exp,
                                     accum_out=esum[:])
                e_h = small.tile([128, LT], F16, tag="eh")
                nc.vector.tensor_copy(e_h[:], e[:])
                # Z = sum(esum) across partitions; rzb = (1/Z) on all parts
                zps = ps_z.tile([1, 1], F32)
                nc.tensor.matmul(zps[:], lhsT=esum[:], rhs=ones_k[:],
                                 start=True, stop=True)
                rz = small.tile([1, 1], F32, tag="rz")
                nc.vector.reciprocal(rz[:], zps[:])
                rzbps = ps_b.tile([128, 1], F32)
                nc.tensor.matmul(rzbps[:], lhsT=ones_m[:], rhs=rz[:],
                                 start=True, stop=True)
                rzb = small.tile([128, 1], F32, tag="rzb")
                nc.vector.tensor_copy(rzb[:], rzbps[:])

                # ---- score output: transpose e, scale by 1/Z on the way ----
                scps = ps_t.tile([LT, 128], F32, tag="tr")
                nc.tensor.transpose(scps[:], e[:], identity[:])
                sc_sb = small.tile([LT, 128], F32, tag="sc")
                nc.scalar.activation(out=sc_sb[:], in_=scps[:], func=AF.Copy,
                                     scale=rzb[0:LT, :])
                nc.gpsimd.dma_start(
                    out=score[b:b + 1, :].rearrange("o (t p) -> (o t) p", p=128),
                    in_=sc_sb[:])

                # ---- wv[b, hc] via PE: v chunks stationary, e col moving ---
                # (fp32 moving operands stream at 1/4 rate; keep them [128,1])
                for c in range(HC):
                    cols = slice(c * 128, (c + 1) * 128)
                    wvc = ps_wvc.tile([128, 1], F32)
                    for t_idx in range(LT):
                        nc.tensor.matmul(
                            wvc[:],
                            lhsT=vts[t_idx // VXT][:, t_idx % VXT, cols],
                            rhs=e_h[:, t_idx:t_idx + 1],
                            start=(t_idx == 0), stop=(t_idx == LT - 1))
                    # evacuate with the 1/Z scale folded in
                    nc.vector.tensor_scalar_mul(
                        wvt_sb[:, c * BPC + b:c * BPC + b + 1],
                        in0=wvc[:], scalar1=rzb[:])

            # ---- result = wv @ W.T: wv block stationary (4-col LDW), fp16
            # wt moving at 1 col/cycle -> lands directly as [BPC, H] ----------
            for c in range(HC):
                nc.sync.dma_start(out=wt_sb[:, c * H:(c + 1) * H],
                                  in_=wt[c * 128:(c + 1) * 128, :])
            rps = ps_res.tile([BPC, H], F32)
            for hc in range(HC):
                for hh in range(2):
                    cols = slice(hh * 512, (hh + 1) * 512)
                    nc.tensor.matmul(
                        rps[:, cols],
                        lhsT=wvt_sb[:, hc * BPC:(hc + 1) * BPC],
                        rhs=wt_sb[:, hc * H + hh * 512: hc * H + (hh + 1) * 512],
                        start=(hc == 0), stop=(hc == HC - 1))
            nc.vector.tensor_copy(res_sb[:], rps[:])
            nc.gpsimd.dma_start(out=result[:, :], in_=res_sb[:])

    _split_multi_waits(nc)
    return nc


_NC_CACHE = None


def _get_nc():
    global _NC_CACHE
    if _NC_CACHE is None:
        _NC_CACHE = build_nc()
    return _NC_CACHE


def kernel(query, key, value, W, mlp_w, mlp_b):
    # q/k/v travel to the device as fp16 (|x| ~ N(0,1), well inside fp16
    # range; adds ~3e-4 scale-relative absmax error, halves the DMA traffic
    # that dominates this kernel).  All accumulation stays fp32 on device.
    query = np.ascontiguousarray(np.asarray(query).astype(np.float16))
    key = np.ascontiguousarray(np.asarray(key).astype(np.float16))
    value = np.ascontiguousarray(np.asarray(value).astype(np.float16))
    W = np.ascontiguousarray(np.asarray(W, dtype=np.float32))
    mlp_w = np.asarray(mlp_w, dtype=np.float32)
    mlp_b = np.asarray(mlp_b, dtype=np.float32)

    # Host-side input prep (O(H^2), ~0.01% of the device work)
    W64 = W.astype(np.float64)
    u1 = (mlp_w[:, :H].astype(np.float64) @ W64).astype(np.float32)  # [1, H]
    u2 = (mlp_w[:, H:].astype(np.float64) @ W64).astype(np.float32)  # [1, H]
    u12 = np.ascontiguousarray(np.concatenate([u1, u2], axis=1))     # [1, 2H]
    wt = np.ascontiguousarray(W.T).astype(np.float16)                # [H, H]
    bias = mlp_b.reshape(1, 1)

    in_maps = []
    for i in range(NCORES):
        s = slice(i * BPC, (i + 1) * BPC)
        in_maps.append({
            "q": query[s], "k": key[s], "v": value[s],
            "wt": wt, "u12": u12, "bias": bias,
        })

    res = run_bass_kernel_spmd(_get_nc(), in_maps, core_ids=list(range(NCORES)))

    result = np.concatenate([r["result"] for r in res.results], axis=0)
    score = np.concatenate([r["score"] for r in res.results], axis=0)
    return result, score.reshape(B, L, 1)
